# revision 7
# baseline (speedup 1.0000x reference)
"""Trainium2 Bass kernel for nn_FFNwMoE (MoE FFN with top-2 routing + shared expert).

Strategy (expert-parallel sparse dispatch, host-side routing):
  - Host computes router logits/softmax/top-2 (jax on CPU, bit-matching the
    reference) plus the aux load-balancing loss.
  - Tokens are gathered per expert on the host. Core e processes expert e's
    tokens (padded to capacity CA) with expert-e weights, plus a static 1/8
    slice of all tokens (CB=1024) with the shared-expert weights.
  - On-device per core: swiglu via fp32r matmuls (full PE rate, ~FP22
    precision): aT/bT = W1/W3 contraction over d, h = silu(a)*b,
    y = hT.T @ W2T accumulated over h-tiles, scaled by the combine weight.
  - Host scatter-adds the per-core outputs back into the full [T, D] output.

All heavy FLOPs (3 matmuls x (2*T top-2 assignments + T shared)) run on the
8 NeuronCores; the host only does O(T*E) routing math and data movement.
All DRAM inputs are host-pre-tiled so DMA descriptors are >=16KB-contiguous
per partition.
"""
import sys

if '/opt/trn_rl_repo' not in sys.path:
    sys.path.insert(0, '/opt/trn_rl_repo')

from contextlib import ExitStack

import numpy as np

import concourse.bass as bass  # noqa: F401  (bass types used via tile/bacc)
import concourse.mybir as mybir
import concourse.tile as tile
from concourse import bacc
from concourse.bass_utils import run_bass_kernel_spmd

F32R = mybir.dt.float32r
F32 = mybir.dt.float32
AF = mybir.ActivationFunctionType

# Problem constants (hardcoded per spec nn_FFNwMoE_74380243632567)
B, S, D = 4, 2048, 2048
E, TOPK, H, SHARED = 8, 2, 1368, 1
AUX_COEF = 0.01
T = B * S                      # 8192 tokens
ND = D // 128                  # 16 d-tiles
NH = (H + 127) // 128          # 11 h-tiles (H padded 1368 -> 1408)
HPAD = NH * 128
NDC = D // 512                 # 4 output d-chunks
NCORES = 8
CB = T // NCORES               # shared-expert slice per core
TBMAX = 1024


def _block_list(CA, CB):
    blocks = []
    for ph, (start, size) in enumerate(((0, CA), (CA, CB))):
        off = start
        while off < start + size:
            TB = min(TBMAX, start + size - off)
            blocks.append((off, TB, ph))
            off += TB
    # Small (tail) blocks first: they need a full weight-set stream for little
    # PE work, so schedule them where DMA has slack and startup latency is low.
    blocks.sort(key=lambda b: (b[2], b[1]))
    out = []
    flat_off = 0
    for (t0, TB, ph) in blocks:
        out.append((t0, TB, ph, flat_off))
        flat_off += 128 * ND * TB
    return out


def _build_moe_kernel(CA, CB):
    """One SPMD Bass program; per-core data arrives via in_maps."""
    CTOT = CA + CB
    NTTG = CTOT // 128
    nc = bacc.Bacc("TRN2", target_bir_lowering=False, debug=False,
                   num_devices=NCORES)

    # xt: block-contiguous flat layout; per block [128, ND, TB] with
    # partition-major contiguity (per-partition run = ND*TB*4 bytes).
    xt = nc.dram_tensor("xt", [128 * ND * CTOT], F32R, kind="ExternalInput").ap()
    w13A = nc.dram_tensor("w13A", [NH, 128, ND, 256], F32R, kind="ExternalInput").ap()
    w2A = nc.dram_tensor("w2A", [NDC, 128, NH, 512], F32R, kind="ExternalInput").ap()
    w13B = nc.dram_tensor("w13B", [NH, 128, ND, 256], F32R, kind="ExternalInput").ap()
    w2B = nc.dram_tensor("w2B", [NDC, 128, NH, 512], F32R, kind="ExternalInput").ap()
    cw = nc.dram_tensor("cw", [128, NTTG], F32, kind="ExternalInput").ap()
    y = nc.dram_tensor("y", [CTOT, D], F32, kind="ExternalOutput").ap()

    blocks = _block_list(CA, CB)

    with tile.TileContext(nc) as tc, ExitStack() as ctx:
        xt_pool = ctx.enter_context(tc.tile_pool(name="xtp", bufs=1))
        h_pool = ctx.enter_context(tc.tile_pool(name="hp", bufs=1))
        w13_pool = ctx.enter_context(tc.tile_pool(name="w13p", bufs=5))
        w2_pool = ctx.enter_context(tc.tile_pool(name="w2p", bufs=2))
        sil_pool = ctx.enter_context(tc.tile_pool(name="silp", bufs=2))
        y_pool = ctx.enter_context(tc.tile_pool(name="yp", bufs=2))
        cw_pool = ctx.enter_context(tc.tile_pool(name="cwp", bufs=1))
        psa_pool = ctx.enter_context(tc.tile_pool(name="psa", bufs=3, space="PSUM"))
        psb_pool = ctx.enter_context(tc.tile_pool(name="psb", bufs=3, space="PSUM"))
        psy_pool = ctx.enter_context(tc.tile_pool(name="psy", bufs=2, space="PSUM"))

        cw_all = cw_pool.tile([128, NTTG], F32, tag="cw")
        nc.sync.dma_start(cw_all[:], cw)

        for (t0, TB, ph, flat_off) in blocks:
            w13X = w13A if ph == 0 else w13B
            w2X = w2A if ph == 0 else w2B
            ntt = TB // 128
            subs = []
            off = 0
            while off < TB:
                subs.append((off, min(512, TB - off)))
                off += 512

            xt_sb = xt_pool.tile([128, ND, TBMAX], F32R, tag="xt")
            xt_blk = xt[flat_off:flat_off + 128 * ND * TB].rearrange(
                "(p kd t) -> p kd t", p=128, kd=ND)
            half = ND // 2
            nc.sync.dma_start(xt_sb[:, :half, :TB], xt_blk[:, :half, :])
            nc.sync.dma_start(xt_sb[:, half:, :TB], xt_blk[:, half:, :])

            h_sb = h_pool.tile([128, NH, TBMAX], F32R, tag="hsb")

            # phase 1: aT/bT[h, t] accumulation over d-tiles; h = silu(a)*b
            for h in range(NH):
                w13_lo = w13_pool.tile([128, ND // 2, 256], F32R, tag="w13")
                w13_hi = w13_pool.tile([128, ND // 2, 256], F32R, tag="w13")
                nc.sync.dma_start(w13_lo[:], w13X[h, :, :half, :])
                nc.sync.dma_start(w13_hi[:], w13X[h, :, half:, :])

                def w13s(kd, j0, j1):
                    t = w13_lo if kd < half else w13_hi
                    return t[:, kd % half, j0:j1]

                for (so, sw) in subs:
                    psa = psa_pool.tile([128, 512], F32, tag="psa")
                    psb = psb_pool.tile([128, 512], F32, tag="psb")
                    for kd in range(ND):
                        nc.tensor.matmul(psa[:, :sw], w13s(kd, 0, 128),
                                         xt_sb[:, kd, so:so + sw],
                                         start=(kd == 0), stop=(kd == ND - 1))
                    for kd in range(ND):
                        nc.tensor.matmul(psb[:, :sw], w13s(kd, 128, 256),
                                         xt_sb[:, kd, so:so + sw],
                                         start=(kd == 0), stop=(kd == ND - 1))
                    sil = sil_pool.tile([128, 512], F32, tag="sil")
                    nc.scalar.activation(sil[:, :sw], psa[:, :sw], AF.Sigmoid)
                    nc.vector.tensor_mul(sil[:, :sw], sil[:, :sw], psa[:, :sw])
                    nc.vector.tensor_mul(h_sb[:, h, so:so + sw], sil[:, :sw],
                                         psb[:, :sw])

            # phase 3: y[t, d] = hT.T @ w2T over h-tiles, scaled by cw[t]
            for dc in range(NDC):
                w2_sb = w2_pool.tile([128, NH, 512], F32R, tag="w2")
                hh = NH // 2
                nc.sync.dma_start(w2_sb[:, :hh, :], w2X[dc, :, :hh, :])
                nc.sync.dma_start(w2_sb[:, hh:, :], w2X[dc, :, hh:, :])
                for tt in range(ntt):
                    psy = psy_pool.tile([128, 512], F32, tag="psy")
                    for h in range(NH):
                        nc.tensor.matmul(psy[:],
                                         h_sb[:, h, tt * 128:(tt + 1) * 128],
                                         w2_sb[:, h, :],
                                         start=(h == 0), stop=(h == NH - 1))
                    y_sb = y_pool.tile([128, 512], F32, tag="y")
                    ttg = t0 // 128 + tt
                    nc.vector.tensor_scalar_mul(y_sb[:], psy[:],
                                                cw_all[:, ttg:ttg + 1])
                    nc.sync.dma_start(
                        y[t0 + tt * 128:t0 + (tt + 1) * 128,
                          dc * 512:(dc + 1) * 512], y_sb[:])

    nc.compile()
    return nc


_KERNEL_CACHE = {}


def _get_kernel(CA):
    key = (CA, CB)
    if key not in _KERNEL_CACHE:
        _KERNEL_CACHE[key] = _build_moe_kernel(CA, CB)
    return _KERNEL_CACHE[key]


# ---------------- host-side packing ----------------

def _pack_w13(w1, w3):
    """w1, w3: [H, D] -> [NH, 128, ND, 256] with w13[h,p,kd,j]=w1p[h*128+j, kd*128+p]."""
    w1p = np.zeros((HPAD, D), np.float32)
    w1p[:H] = w1
    w3p = np.zeros((HPAD, D), np.float32)
    w3p[:H] = w3
    a = w1p.reshape(NH, 128, ND, 128).transpose(0, 3, 2, 1)
    b = w3p.reshape(NH, 128, ND, 128).transpose(0, 3, 2, 1)
    return np.ascontiguousarray(np.concatenate([a, b], axis=3))


def _pack_w2(w2):
    """w2: [D, H] -> [NDC, 128, NH, 512] with w2t[dc,p,h,j] = w2[dc*512+j, h*128+p]."""
    w2tp = np.zeros((HPAD, D), np.float32)
    w2tp[:H] = w2.T
    return np.ascontiguousarray(
        w2tp.reshape(NH, 128, NDC, 512).transpose(2, 1, 0, 3))


def _pack_xt(xcols, CA):
    """xcols: [CTOT, D] -> block-contiguous flat [128*ND*CTOT]:
    per block, layout [p, kd, t_local] with xt[p,kd,t]=xcols[t, kd*128+p]."""
    CTOT = xcols.shape[0]
    out = np.empty(128 * ND * CTOT, np.float32)
    for (t0, TB, _ph, flat_off) in _block_list(CA, CTOT - CA):
        blk = xcols[t0:t0 + TB].reshape(TB, ND, 128).transpose(2, 1, 0)
        out[flat_off:flat_off + 128 * ND * TB] = blk.reshape(-1)
    return out


def _routing(xf, router_w):
    """Replicate the reference's router math on CPU via jax (bit-matching ops).

    Returns (topk_idx [T, K] int, cw [T, E] f32, aux_loss f32 scalar).
    """
    import jax
    import jax.numpy as jnp
    cpu = jax.devices('cpu')[0]
    with jax.default_device(cpu):
        xj = jnp.asarray(xf)
        rwj = jnp.asarray(router_w)
        logits = xj @ rwj.T
        probs = jax.nn.softmax(logits, axis=-1)
        _, topk_idx = jax.lax.top_k(logits, TOPK)
        topk_p, _ = jax.lax.top_k(probs, TOPK)
        density = jax.nn.one_hot(topk_idx[:, 0], E, dtype=jnp.float32).mean(0)
        aux_loss = AUX_COEF * jnp.sum(density * probs.mean(0)) * E
        topk_idx = np.asarray(topk_idx)
        topk_p = np.asarray(topk_p)
        aux_loss = np.asarray(aux_loss)
    cwf = np.zeros((xf.shape[0], E), np.float32)
    np.add.at(cwf, (np.arange(xf.shape[0])[:, None], topk_idx), topk_p)
    return topk_idx, cwf, aux_loss


def kernel(x, router_w, w1, w2, w3, sw1, sw2, sw3, _run_opts=None):
    x = np.asarray(x, dtype=np.float32)
    router_w = np.asarray(router_w, dtype=np.float32)
    w1 = np.asarray(w1, dtype=np.float32)
    w2 = np.asarray(w2, dtype=np.float32)
    w3 = np.asarray(w3, dtype=np.float32)
    sw1 = np.asarray(sw1, dtype=np.float32)
    sw2 = np.asarray(sw2, dtype=np.float32)
    sw3 = np.asarray(sw3, dtype=np.float32)

    xf = x.reshape(T, D)
    topk_idx, cwf, aux_loss = _routing(xf, router_w)

    idx = [np.nonzero((topk_idx == e).any(axis=1))[0] for e in range(E)]
    counts = np.array([len(i) for i in idx])
    CA = max(256, int(-(-counts.max() // 256)) * 256)
    CTOT = CA + CB

    nc = _get_kernel(CA)

    w13B = _pack_w13(sw1[0], sw3[0])
    w2B = _pack_w2(sw2[0])

    in_maps = []
    for e in range(E):
        xcols = np.zeros((CTOT, D), np.float32)
        xcols[:counts[e]] = xf[idx[e]]
        xcols[CA:] = xf[e * CB:(e + 1) * CB]
        cw_col = np.zeros(CTOT, np.float32)
        cw_col[:counts[e]] = cwf[idx[e], e]
        cw_col[CA:] = 1.0
        in_maps.append({
            "xt": _pack_xt(xcols, CA),
            "w13A": _pack_w13(w1[e], w3[e]),
            "w2A": _pack_w2(w2[e]),
            "w13B": w13B,
            "w2B": w2B,
            "cw": np.ascontiguousarray(cw_col.reshape(CTOT // 128, 128).T),
        })

    run_opts = _run_opts or {}
    res = run_bass_kernel_spmd(nc, in_maps, core_ids=list(range(NCORES)),
                               **run_opts)

    out = np.zeros((T, D), np.float32)
    for e in range(E):
        ye = res.results[e]["y"]
        out[idx[e]] += ye[:counts[e]]
        out[e * CB:(e + 1) * CB] += ye[CA:]

    if run_opts:
        kernel._last_result = res
    return out.reshape(B, S, D), aux_loss


# revision 9
# speedup vs baseline: 1.1183x; 1.1183x over previous
"""Trainium2 Bass kernel for nn_FFNwMoE (MoE FFN with top-2 routing + shared expert).

Strategy (expert-parallel sparse dispatch, host-side routing):
  - Host computes router logits/softmax/top-2 (jax on CPU, bit-matching the
    reference) plus the aux load-balancing loss.
  - Tokens are gathered per expert on the host. Core e processes expert e's
    tokens (padded to capacity CA) with expert-e weights, plus a static 1/8
    slice of all tokens (CB=1024) with the shared-expert weights.
  - On-device per core: swiglu via fp32r matmuls (full PE rate, ~FP22
    precision): aT/bT = W1/W3 contraction over d, h = silu(a)*b,
    y = hT.T @ W2T accumulated over h-tiles, scaled by the combine weight.
  - Host scatter-adds the per-core outputs back into the full [T, D] output.

All heavy FLOPs (3 matmuls x (2*T top-2 assignments + T shared)) run on the
8 NeuronCores; the host only does O(T*E) routing math and data movement.
All DRAM inputs are host-pre-tiled so DMA descriptors are >=16KB-contiguous
per partition.
"""
import sys

if '/opt/trn_rl_repo' not in sys.path:
    sys.path.insert(0, '/opt/trn_rl_repo')

from contextlib import ExitStack

import numpy as np

try:
    # bass_utils imports this on the trace path; the module is absent on some
    # images, which would turn an optional profile into a hard crash. Provide
    # a null hook so tracing degrades gracefully instead.
    import antenv.axon_hooks  # noqa: F401
except ImportError:
    import types as _types

    _m = _types.ModuleType('antenv.axon_hooks')
    _m._hook = None
    _m.set_axon_ntff_profile_hook = lambda h: setattr(_m, '_hook', h)
    _m.get_axon_ntff_profile_hook = lambda: _m._hook
    sys.modules['antenv.axon_hooks'] = _m

import concourse.bass as bass  # noqa: F401  (bass types used via tile/bacc)
import concourse.mybir as mybir
import concourse.tile as tile
from concourse import bacc
from concourse.bass_utils import run_bass_kernel_spmd

F32R = mybir.dt.float32r
F32 = mybir.dt.float32
AF = mybir.ActivationFunctionType

# Problem constants (hardcoded per spec nn_FFNwMoE_74380243632567)
B, S, D = 4, 2048, 2048
E, TOPK, H, SHARED = 8, 2, 1368, 1
AUX_COEF = 0.01
T = B * S                      # 8192 tokens
ND = D // 128                  # 16 d-tiles
NH = (H + 127) // 128          # 11 h-tiles (H padded 1368 -> 1408)
HPAD = NH * 128
NDC = D // 512                 # 4 output d-chunks
NCORES = 8
CB = T // NCORES               # shared-expert slice per core
TBMAX = 1280


def _block_list(CA, CB):
    blocks = []
    for ph, (start, size) in enumerate(((0, CA), (CA, CB))):
        off = start
        while off < start + size:
            rem = start + size - off
            TB = 1024 if rem > TBMAX else rem
            blocks.append((off, TB, ph))
            off += TB
    # Big blocks first within each phase: boundaries then always sit after a
    # long phase-3 window that hides the next block's x/weight prefetch.
    blocks.sort(key=lambda b: (b[2], -b[1]))
    out = []
    flat_off = 0
    for (t0, TB, ph) in blocks:
        out.append((t0, TB, ph, flat_off))
        flat_off += 128 * ND * TB
    return out


def _build_moe_kernel(CA, CB):
    """One SPMD Bass program; per-core data arrives via in_maps."""
    CTOT = CA + CB
    NTTG = CTOT // 128
    nc = bacc.Bacc("TRN2", target_bir_lowering=False, debug=False,
                   num_devices=NCORES)

    # xt: block-contiguous flat layout; per block [128, ND, TB] with
    # partition-major contiguity (per-partition run = ND*TB*4 bytes).
    xt = nc.dram_tensor("xt", [128 * ND * CTOT], F32R, kind="ExternalInput").ap()
    w13A = nc.dram_tensor("w13A", [NH, 128, ND, 256], F32R, kind="ExternalInput").ap()
    w2A = nc.dram_tensor("w2A", [NDC, 128, NH, 512], F32R, kind="ExternalInput").ap()
    w13B = nc.dram_tensor("w13B", [NH, 128, ND, 256], F32R, kind="ExternalInput").ap()
    w2B = nc.dram_tensor("w2B", [NDC, 128, NH, 512], F32R, kind="ExternalInput").ap()
    cw = nc.dram_tensor("cw", [128, NTTG], F32, kind="ExternalInput").ap()
    y = nc.dram_tensor("y", [CTOT, D], F32, kind="ExternalOutput").ap()

    blocks = _block_list(CA, CB)

    with tile.TileContext(nc) as tc, ExitStack() as ctx:
        xt_pool = ctx.enter_context(tc.tile_pool(name="xtp", bufs=1))
        h_pool = ctx.enter_context(tc.tile_pool(name="hp", bufs=1))
        w13_pool = ctx.enter_context(tc.tile_pool(name="w13p", bufs=3))
        w2_pool = ctx.enter_context(tc.tile_pool(name="w2p", bufs=2))
        y_pool = ctx.enter_context(tc.tile_pool(name="yp", bufs=2))
        cw_pool = ctx.enter_context(tc.tile_pool(name="cwp", bufs=1))
        psa_pool = ctx.enter_context(tc.tile_pool(name="psa", bufs=3, space="PSUM"))
        psb_pool = ctx.enter_context(tc.tile_pool(name="psb", bufs=3, space="PSUM"))
        psy_pool = ctx.enter_context(tc.tile_pool(name="psy", bufs=2, space="PSUM"))

        cw_all = cw_pool.tile([128, NTTG], F32, tag="cw")
        nc.sync.dma_start(cw_all[:], cw)

        for (t0, TB, ph, flat_off) in blocks:
            w13X = w13A if ph == 0 else w13B
            w2X = w2A if ph == 0 else w2B
            ntt = TB // 128
            subs = []
            off = 0
            while off < TB:
                subs.append((off, min(512, TB - off)))
                off += 512

            xt_sb = xt_pool.tile([128, ND, TBMAX], F32R, tag="xt")
            xt_blk = xt[flat_off:flat_off + 128 * ND * TB].rearrange(
                "(p kd t) -> p kd t", p=128, kd=ND)
            half = ND // 2
            q = ND // 4
            for qi in range(4):
                nc.sync.dma_start(xt_sb[:, qi * q:(qi + 1) * q, :TB],
                                  xt_blk[:, qi * q:(qi + 1) * q, :])

            h_sb = h_pool.tile([128, NH, TBMAX], F32R, tag="hsb")

            # phase 1: aT/bT[h, t] accumulation over d-tiles; h = silu(a)*b
            for h in range(NH):
                w13_lo = w13_pool.tile([128, ND // 2, 256], F32R, tag="w13")
                w13_hi = w13_pool.tile([128, ND // 2, 256], F32R, tag="w13")
                nc.sync.dma_start(w13_lo[:], w13X[h, :, :half, :])
                nc.sync.dma_start(w13_hi[:], w13X[h, :, half:, :])

                def w13s(kd, j0, j1):
                    t = w13_lo if kd < half else w13_hi
                    return t[:, kd % half, j0:j1]

                for (so, sw) in subs:
                    psa = psa_pool.tile([128, 512], F32, tag="psa")
                    psb = psb_pool.tile([128, 512], F32, tag="psb")
                    for kd in range(ND):
                        nc.tensor.matmul(psa[:, :sw], w13s(kd, 0, 128),
                                         xt_sb[:, kd, so:so + sw],
                                         start=(kd == 0), stop=(kd == ND - 1))
                    for kd in range(ND):
                        nc.tensor.matmul(psb[:, :sw], w13s(kd, 128, 256),
                                         xt_sb[:, kd, so:so + sw],
                                         start=(kd == 0), stop=(kd == ND - 1))
                    hs = h_sb[:, h, so:so + sw]
                    nc.scalar.activation(hs, psa[:, :sw], AF.Sigmoid)
                    nc.vector.tensor_mul(hs, hs, psa[:, :sw])
                    nc.vector.tensor_mul(hs, hs, psb[:, :sw])

            # phase 3: y[t, d] = hT.T @ w2T over h-tiles, scaled by cw[t]
            for dc in range(NDC):
                w2_sb = w2_pool.tile([128, NH, 512], F32R, tag="w2")
                hh = NH // 2
                nc.sync.dma_start(w2_sb[:, :hh, :], w2X[dc, :, :hh, :])
                nc.sync.dma_start(w2_sb[:, hh:, :], w2X[dc, :, hh:, :])
                for tt in range(ntt):
                    psy = psy_pool.tile([128, 512], F32, tag="psy")
                    for h in range(NH):
                        nc.tensor.matmul(psy[:],
                                         h_sb[:, h, tt * 128:(tt + 1) * 128],
                                         w2_sb[:, h, :],
                                         start=(h == 0), stop=(h == NH - 1))
                    y_sb = y_pool.tile([128, 512], F32, tag="y")
                    ttg = t0 // 128 + tt
                    nc.vector.tensor_scalar_mul(y_sb[:], psy[:],
                                                cw_all[:, ttg:ttg + 1])
                    nc.sync.dma_start(
                        y[t0 + tt * 128:t0 + (tt + 1) * 128,
                          dc * 512:(dc + 1) * 512], y_sb[:])

    nc.compile()
    return nc


_KERNEL_CACHE = {}


def _get_kernel(CA):
    key = (CA, CB)
    if key not in _KERNEL_CACHE:
        _KERNEL_CACHE[key] = _build_moe_kernel(CA, CB)
    return _KERNEL_CACHE[key]


# ---------------- host-side packing ----------------

def _pack_w13(w1, w3):
    """w1, w3: [H, D] -> [NH, 128, ND, 256] with w13[h,p,kd,j]=w1p[h*128+j, kd*128+p]."""
    w1p = np.zeros((HPAD, D), np.float32)
    w1p[:H] = w1
    w3p = np.zeros((HPAD, D), np.float32)
    w3p[:H] = w3
    a = w1p.reshape(NH, 128, ND, 128).transpose(0, 3, 2, 1)
    b = w3p.reshape(NH, 128, ND, 128).transpose(0, 3, 2, 1)
    return np.ascontiguousarray(np.concatenate([a, b], axis=3))


def _pack_w2(w2):
    """w2: [D, H] -> [NDC, 128, NH, 512] with w2t[dc,p,h,j] = w2[dc*512+j, h*128+p]."""
    w2tp = np.zeros((HPAD, D), np.float32)
    w2tp[:H] = w2.T
    return np.ascontiguousarray(
        w2tp.reshape(NH, 128, NDC, 512).transpose(2, 1, 0, 3))


def _pack_xt(xcols, CA):
    """xcols: [CTOT, D] -> block-contiguous flat [128*ND*CTOT]:
    per block, layout [p, kd, t_local] with xt[p,kd,t]=xcols[t, kd*128+p]."""
    CTOT = xcols.shape[0]
    out = np.empty(128 * ND * CTOT, np.float32)
    for (t0, TB, _ph, flat_off) in _block_list(CA, CTOT - CA):
        blk = xcols[t0:t0 + TB].reshape(TB, ND, 128).transpose(2, 1, 0)
        out[flat_off:flat_off + 128 * ND * TB] = blk.reshape(-1)
    return out


def _routing(xf, router_w):
    """Replicate the reference's router math on CPU via jax (bit-matching ops).

    Returns (topk_idx [T, K] int, cw [T, E] f32, aux_loss f32 scalar).
    """
    import jax
    import jax.numpy as jnp
    cpu = jax.devices('cpu')[0]
    with jax.default_device(cpu):
        xj = jnp.asarray(xf)
        rwj = jnp.asarray(router_w)
        logits = xj @ rwj.T
        probs = jax.nn.softmax(logits, axis=-1)
        _, topk_idx = jax.lax.top_k(logits, TOPK)
        topk_p, _ = jax.lax.top_k(probs, TOPK)
        density = jax.nn.one_hot(topk_idx[:, 0], E, dtype=jnp.float32).mean(0)
        aux_loss = AUX_COEF * jnp.sum(density * probs.mean(0)) * E
        topk_idx = np.asarray(topk_idx)
        topk_p = np.asarray(topk_p)
        aux_loss = np.asarray(aux_loss)
    cwf = np.zeros((xf.shape[0], E), np.float32)
    np.add.at(cwf, (np.arange(xf.shape[0])[:, None], topk_idx), topk_p)
    return topk_idx, cwf, aux_loss


def kernel(x, router_w, w1, w2, w3, sw1, sw2, sw3, _run_opts=None):
    x = np.asarray(x, dtype=np.float32)
    router_w = np.asarray(router_w, dtype=np.float32)
    w1 = np.asarray(w1, dtype=np.float32)
    w2 = np.asarray(w2, dtype=np.float32)
    w3 = np.asarray(w3, dtype=np.float32)
    sw1 = np.asarray(sw1, dtype=np.float32)
    sw2 = np.asarray(sw2, dtype=np.float32)
    sw3 = np.asarray(sw3, dtype=np.float32)

    xf = x.reshape(T, D)
    topk_idx, cwf, aux_loss = _routing(xf, router_w)

    idx = [np.nonzero((topk_idx == e).any(axis=1))[0] for e in range(E)]
    counts = np.array([len(i) for i in idx])
    CA = max(256, int(-(-counts.max() // 256)) * 256)
    CTOT = CA + CB

    nc = _get_kernel(CA)

    w13B = _pack_w13(sw1[0], sw3[0])
    w2B = _pack_w2(sw2[0])

    in_maps = []
    for e in range(E):
        xcols = np.zeros((CTOT, D), np.float32)
        xcols[:counts[e]] = xf[idx[e]]
        xcols[CA:] = xf[e * CB:(e + 1) * CB]
        cw_col = np.zeros(CTOT, np.float32)
        cw_col[:counts[e]] = cwf[idx[e], e]
        cw_col[CA:] = 1.0
        in_maps.append({
            "xt": _pack_xt(xcols, CA),
            "w13A": _pack_w13(w1[e], w3[e]),
            "w2A": _pack_w2(w2[e]),
            "w13B": w13B,
            "w2B": w2B,
            "cw": np.ascontiguousarray(cw_col.reshape(CTOT // 128, 128).T),
        })

    run_opts = _run_opts or {}
    res = run_bass_kernel_spmd(nc, in_maps, core_ids=list(range(NCORES)),
                               **run_opts)

    out = np.zeros((T, D), np.float32)
    for e in range(E):
        ye = res.results[e]["y"]
        out[idx[e]] += ye[:counts[e]]
        out[e * CB:(e + 1) * CB] += ye[CA:]

    if run_opts:
        kernel._last_result = res
    return out.reshape(B, S, D), aux_loss


# revision 12
# speedup vs baseline: 1.1262x; 1.0071x over previous
"""Trainium2 Bass kernel for nn_FFNwMoE (MoE FFN with top-2 routing + shared expert).

Strategy (expert-parallel sparse dispatch, host-side routing):
  - Host computes router logits/softmax/top-2 (jax on CPU, bit-matching the
    reference) plus the aux load-balancing loss.
  - Tokens are gathered per expert on the host. Core e processes expert e's
    tokens (padded to capacity CA) with expert-e weights, plus a static 1/8
    slice of all tokens (CB=1024) with the shared-expert weights.
  - On-device per core: swiglu via fp32r matmuls (full PE rate, ~FP22
    precision): aT/bT = W1/W3 contraction over d, h = silu(a)*b,
    y = hT.T @ W2T accumulated over h-tiles, scaled by the combine weight.
  - Host scatter-adds the per-core outputs back into the full [T, D] output.

All heavy FLOPs (3 matmuls x (2*T top-2 assignments + T shared)) run on the
8 NeuronCores; the host only does O(T*E) routing math and data movement.
All DRAM inputs are host-pre-tiled so DMA descriptors are >=16KB-contiguous
per partition.
"""
import sys

if '/opt/trn_rl_repo' not in sys.path:
    sys.path.insert(0, '/opt/trn_rl_repo')

from contextlib import ExitStack

import numpy as np

try:
    # bass_utils imports this on the trace path; the module is absent on some
    # images, which would turn an optional profile into a hard crash. Provide
    # a null hook so tracing degrades gracefully instead.
    import antenv.axon_hooks  # noqa: F401
except ImportError:
    import types as _types

    _m = _types.ModuleType('antenv.axon_hooks')
    _m._hook = None
    _m.set_axon_ntff_profile_hook = lambda h: setattr(_m, '_hook', h)
    _m.get_axon_ntff_profile_hook = lambda: _m._hook
    sys.modules['antenv.axon_hooks'] = _m

import concourse.bass as bass  # noqa: F401  (bass types used via tile/bacc)
import concourse.mybir as mybir
import concourse.tile as tile
from concourse import bacc
from concourse.bass_utils import run_bass_kernel_spmd

F32R = mybir.dt.float32r
F32 = mybir.dt.float32
AF = mybir.ActivationFunctionType

# Problem constants (hardcoded per spec nn_FFNwMoE_74380243632567)
B, S, D = 4, 2048, 2048
E, TOPK, H, SHARED = 8, 2, 1368, 1
AUX_COEF = 0.01
T = B * S                      # 8192 tokens
ND = D // 128                  # 16 d-tiles
NH = (H + 127) // 128          # 11 h-tiles (H padded 1368 -> 1408)
HPAD = NH * 128
NDC = D // 512                 # 4 output d-chunks
NCORES = 8
CB = T // NCORES               # shared-expert slice per core
TBMAX = 1280


def _block_list(CA, CB):
    blocks = []
    for ph, (start, size) in enumerate(((0, CA), (CA, CB))):
        off = start
        while off < start + size:
            rem = start + size - off
            TB = 1024 if rem > TBMAX else rem
            blocks.append((off, TB, ph))
            off += TB
    # Big blocks first within each phase: boundaries then always sit after a
    # long phase-3 window that hides the next block's x/weight prefetch.
    blocks.sort(key=lambda b: (b[2], -b[1]))
    out = []
    flat_off = 0
    for (t0, TB, ph) in blocks:
        out.append((t0, TB, ph, flat_off))
        flat_off += 128 * ND * TB
    return out


def _build_moe_kernel(CA, CB):
    """One SPMD Bass program; per-core data arrives via in_maps."""
    CTOT = CA + CB
    NTTG = CTOT // 128
    nc = bacc.Bacc("TRN2", target_bir_lowering=False, debug=False,
                   num_devices=NCORES)

    # xt: block-contiguous flat layout; per block [128, ND, TB] with
    # partition-major contiguity (per-partition run = ND*TB*4 bytes).
    xt = nc.dram_tensor("xt", [128 * ND * CTOT], F32R, kind="ExternalInput").ap()
    w13A = nc.dram_tensor("w13A", [NH, 128, ND, 256], F32R, kind="ExternalInput").ap()
    w2A = nc.dram_tensor("w2A", [NDC, 128, NH, 512], F32R, kind="ExternalInput").ap()
    w13B = nc.dram_tensor("w13B", [NH, 128, ND, 256], F32R, kind="ExternalInput").ap()
    w2B = nc.dram_tensor("w2B", [NDC, 128, NH, 512], F32R, kind="ExternalInput").ap()
    cw = nc.dram_tensor("cw", [128, NTTG], F32, kind="ExternalInput").ap()
    y = nc.dram_tensor("y", [CTOT, D], F32, kind="ExternalOutput").ap()

    blocks = _block_list(CA, CB)

    with tile.TileContext(nc) as tc, ExitStack() as ctx:
        xt_pool = ctx.enter_context(tc.tile_pool(name="xtp", bufs=1))
        h_pool = ctx.enter_context(tc.tile_pool(name="hp", bufs=1))
        w13_pool = ctx.enter_context(tc.tile_pool(name="w13p", bufs=3))
        w2_pool = ctx.enter_context(tc.tile_pool(name="w2p", bufs=2))
        y_pool = ctx.enter_context(tc.tile_pool(name="yp", bufs=2))
        cw_pool = ctx.enter_context(tc.tile_pool(name="cwp", bufs=1))
        psa_pool = ctx.enter_context(tc.tile_pool(name="psa", bufs=3, space="PSUM"))
        psb_pool = ctx.enter_context(tc.tile_pool(name="psb", bufs=3, space="PSUM"))
        psy_pool = ctx.enter_context(tc.tile_pool(name="psy", bufs=2, space="PSUM"))

        cw_all = cw_pool.tile([128, NTTG], F32, tag="cw")

        first_block = True
        for (t0, TB, ph, flat_off) in blocks:
            w13X = w13A if ph == 0 else w13B
            w2X = w2A if ph == 0 else w2B
            ntt = TB // 128
            subs = []
            off = 0
            while off < TB:
                subs.append((off, min(512, TB - off)))
                off += 512

            xt_sb = xt_pool.tile([128, ND, TBMAX], F32R, tag="xt")
            xt_blk = xt[flat_off:flat_off + 128 * ND * TB].rearrange(
                "(p kd t) -> p kd t", p=128, kd=ND)
            half = ND // 2
            q = ND // 4
            # critical path first: xt quarter 0 + h-tile-0 weights, then the rest
            nc.sync.dma_start(xt_sb[:, :q, :TB], xt_blk[:, :q, :])
            w13_first = (w13_pool.tile([128, ND // 2, 256], F32R, tag="w13",
                                       name="w13_f0"),
                         w13_pool.tile([128, ND // 2, 256], F32R, tag="w13",
                                       name="w13_f1"))
            nc.sync.dma_start(w13_first[0][:], w13X[0, :, :half, :])
            for qi in range(1, 4):
                nc.sync.dma_start(xt_sb[:, qi * q:(qi + 1) * q, :TB],
                                  xt_blk[:, qi * q:(qi + 1) * q, :])
            nc.sync.dma_start(w13_first[1][:], w13X[0, :, half:, :])
            if first_block:
                nc.sync.dma_start(cw_all[:], cw)
                first_block = False

            h_sb = h_pool.tile([128, NH, TBMAX], F32R, tag="hsb")

            # phase 1: aT/bT[h, t] accumulation over d-tiles; h = silu(a)*b
            for h in range(NH):
                if h == 0:
                    w13_lo, w13_hi = w13_first
                else:
                    w13_lo = w13_pool.tile([128, ND // 2, 256], F32R, tag="w13")
                    w13_hi = w13_pool.tile([128, ND // 2, 256], F32R, tag="w13")
                    nc.sync.dma_start(w13_lo[:], w13X[h, :, :half, :])
                    nc.sync.dma_start(w13_hi[:], w13X[h, :, half:, :])

                def w13s(kd, j0, j1):
                    t = w13_lo if kd < half else w13_hi
                    return t[:, kd % half, j0:j1]

                for (so, sw) in subs:
                    psa = psa_pool.tile([128, 512], F32, tag="psa")
                    psb = psb_pool.tile([128, 512], F32, tag="psb")
                    for kd in range(ND):
                        nc.tensor.matmul(psa[:, :sw], w13s(kd, 0, 128),
                                         xt_sb[:, kd, so:so + sw],
                                         start=(kd == 0), stop=(kd == ND - 1))
                    for kd in range(ND):
                        nc.tensor.matmul(psb[:, :sw], w13s(kd, 128, 256),
                                         xt_sb[:, kd, so:so + sw],
                                         start=(kd == 0), stop=(kd == ND - 1))
                    hs = h_sb[:, h, so:so + sw]
                    nc.scalar.activation(hs, psa[:, :sw], AF.Sigmoid)
                    nc.vector.tensor_mul(hs, hs, psa[:, :sw])
                    nc.vector.tensor_mul(hs, hs, psb[:, :sw])

            # phase 3: y[t, d] = hT.T @ w2T over h-tiles, scaled by cw[t]
            for dc in range(NDC):
                w2_sb = w2_pool.tile([128, NH, 512], F32R, tag="w2")
                hh = NH // 2
                nc.sync.dma_start(w2_sb[:, :hh, :], w2X[dc, :, :hh, :])
                nc.sync.dma_start(w2_sb[:, hh:, :], w2X[dc, :, hh:, :])
                for tt in range(ntt):
                    psy = psy_pool.tile([128, 512], F32, tag="psy")
                    for h in range(NH):
                        nc.tensor.matmul(psy[:],
                                         h_sb[:, h, tt * 128:(tt + 1) * 128],
                                         w2_sb[:, h, :],
                                         start=(h == 0), stop=(h == NH - 1))
                    y_sb = y_pool.tile([128, 512], F32, tag="y")
                    ttg = t0 // 128 + tt
                    nc.vector.tensor_scalar_mul(y_sb[:], psy[:],
                                                cw_all[:, ttg:ttg + 1])
                    nc.sync.dma_start(
                        y[t0 + tt * 128:t0 + (tt + 1) * 128,
                          dc * 512:(dc + 1) * 512], y_sb[:])

    nc.compile()
    return nc


_KERNEL_CACHE = {}


def _get_kernel(CA):
    key = (CA, CB)
    if key not in _KERNEL_CACHE:
        _KERNEL_CACHE[key] = _build_moe_kernel(CA, CB)
    return _KERNEL_CACHE[key]


# ---------------- host-side packing ----------------

def _pack_w13(w1, w3):
    """w1, w3: [H, D] -> [NH, 128, ND, 256] with w13[h,p,kd,j]=w1p[h*128+j, kd*128+p]."""
    w1p = np.zeros((HPAD, D), np.float32)
    w1p[:H] = w1
    w3p = np.zeros((HPAD, D), np.float32)
    w3p[:H] = w3
    a = w1p.reshape(NH, 128, ND, 128).transpose(0, 3, 2, 1)
    b = w3p.reshape(NH, 128, ND, 128).transpose(0, 3, 2, 1)
    return np.ascontiguousarray(np.concatenate([a, b], axis=3))


def _pack_w2(w2):
    """w2: [D, H] -> [NDC, 128, NH, 512] with w2t[dc,p,h,j] = w2[dc*512+j, h*128+p]."""
    w2tp = np.zeros((HPAD, D), np.float32)
    w2tp[:H] = w2.T
    return np.ascontiguousarray(
        w2tp.reshape(NH, 128, NDC, 512).transpose(2, 1, 0, 3))


def _pack_xt(xcols, CA):
    """xcols: [CTOT, D] -> block-contiguous flat [128*ND*CTOT]:
    per block, layout [p, kd, t_local] with xt[p,kd,t]=xcols[t, kd*128+p]."""
    CTOT = xcols.shape[0]
    out = np.empty(128 * ND * CTOT, np.float32)
    for (t0, TB, _ph, flat_off) in _block_list(CA, CTOT - CA):
        blk = xcols[t0:t0 + TB].reshape(TB, ND, 128).transpose(2, 1, 0)
        out[flat_off:flat_off + 128 * ND * TB] = blk.reshape(-1)
    return out


def _routing(xf, router_w):
    """Replicate the reference's router math on CPU via jax (bit-matching ops).

    Returns (topk_idx [T, K] int, cw [T, E] f32, aux_loss f32 scalar).
    """
    import jax
    import jax.numpy as jnp
    cpu = jax.devices('cpu')[0]
    with jax.default_device(cpu):
        xj = jnp.asarray(xf)
        rwj = jnp.asarray(router_w)
        logits = xj @ rwj.T
        probs = jax.nn.softmax(logits, axis=-1)
        _, topk_idx = jax.lax.top_k(logits, TOPK)
        topk_p, _ = jax.lax.top_k(probs, TOPK)
        density = jax.nn.one_hot(topk_idx[:, 0], E, dtype=jnp.float32).mean(0)
        aux_loss = AUX_COEF * jnp.sum(density * probs.mean(0)) * E
        topk_idx = np.asarray(topk_idx)
        topk_p = np.asarray(topk_p)
        aux_loss = np.asarray(aux_loss)
    cwf = np.zeros((xf.shape[0], E), np.float32)
    np.add.at(cwf, (np.arange(xf.shape[0])[:, None], topk_idx), topk_p)
    return topk_idx, cwf, aux_loss


def kernel(x, router_w, w1, w2, w3, sw1, sw2, sw3, _run_opts=None):
    x = np.asarray(x, dtype=np.float32)
    router_w = np.asarray(router_w, dtype=np.float32)
    w1 = np.asarray(w1, dtype=np.float32)
    w2 = np.asarray(w2, dtype=np.float32)
    w3 = np.asarray(w3, dtype=np.float32)
    sw1 = np.asarray(sw1, dtype=np.float32)
    sw2 = np.asarray(sw2, dtype=np.float32)
    sw3 = np.asarray(sw3, dtype=np.float32)

    xf = x.reshape(T, D)
    topk_idx, cwf, aux_loss = _routing(xf, router_w)

    idx = [np.nonzero((topk_idx == e).any(axis=1))[0] for e in range(E)]
    counts = np.array([len(i) for i in idx])
    CA = max(256, int(-(-counts.max() // 256)) * 256)
    CTOT = CA + CB

    nc = _get_kernel(CA)

    w13B = _pack_w13(sw1[0], sw3[0])
    w2B = _pack_w2(sw2[0])

    in_maps = []
    for e in range(E):
        xcols = np.zeros((CTOT, D), np.float32)
        xcols[:counts[e]] = xf[idx[e]]
        xcols[CA:] = xf[e * CB:(e + 1) * CB]
        cw_col = np.zeros(CTOT, np.float32)
        cw_col[:counts[e]] = cwf[idx[e], e]
        cw_col[CA:] = 1.0
        in_maps.append({
            "xt": _pack_xt(xcols, CA),
            "w13A": _pack_w13(w1[e], w3[e]),
            "w2A": _pack_w2(w2[e]),
            "w13B": w13B,
            "w2B": w2B,
            "cw": np.ascontiguousarray(cw_col.reshape(CTOT // 128, 128).T),
        })

    run_opts = _run_opts or {}
    res = run_bass_kernel_spmd(nc, in_maps, core_ids=list(range(NCORES)),
                               **run_opts)

    out = np.zeros((T, D), np.float32)
    for e in range(E):
        ye = res.results[e]["y"]
        out[idx[e]] += ye[:counts[e]]
        out[e * CB:(e + 1) * CB] += ye[CA:]

    if run_opts:
        kernel._last_result = res
    return out.reshape(B, S, D), aux_loss


# revision 14
# speedup vs baseline: 1.1359x; 1.0086x over previous
"""Trainium2 Bass kernel for nn_FFNwMoE (MoE FFN with top-2 routing + shared expert).

Strategy (expert-parallel sparse dispatch, host-side routing):
  - Host computes router logits/softmax/top-2 (jax on CPU, bit-matching the
    reference) plus the aux load-balancing loss.
  - Tokens are gathered per expert on the host. Core e processes expert e's
    tokens (padded to capacity CA) with expert-e weights, plus a static 1/8
    slice of all tokens (CB=1024) with the shared-expert weights.
  - On-device per core: swiglu via fp32r matmuls (full PE rate, ~FP22
    precision): aT/bT = W1/W3 contraction over d, h = silu(a)*b,
    y = hT.T @ W2T accumulated over h-tiles, scaled by the combine weight.
  - Host scatter-adds the per-core outputs back into the full [T, D] output.

All heavy FLOPs (3 matmuls x (2*T top-2 assignments + T shared)) run on the
8 NeuronCores; the host only does O(T*E) routing math and data movement.
All DRAM inputs are host-pre-tiled so DMA descriptors are >=16KB-contiguous
per partition.
"""
import sys

if '/opt/trn_rl_repo' not in sys.path:
    sys.path.insert(0, '/opt/trn_rl_repo')

from contextlib import ExitStack

import numpy as np

try:
    # bass_utils imports this on the trace path; the module is absent on some
    # images, which would turn an optional profile into a hard crash. Provide
    # a null hook so tracing degrades gracefully instead.
    import antenv.axon_hooks  # noqa: F401
except ImportError:
    import types as _types

    _m = _types.ModuleType('antenv.axon_hooks')
    _m._hook = None
    _m.set_axon_ntff_profile_hook = lambda h: setattr(_m, '_hook', h)
    _m.get_axon_ntff_profile_hook = lambda: _m._hook
    sys.modules['antenv.axon_hooks'] = _m

import concourse.bass as bass  # noqa: F401  (bass types used via tile/bacc)
import concourse.mybir as mybir
import concourse.tile as tile
from concourse import bacc
from concourse.bass_utils import run_bass_kernel_spmd

F32R = mybir.dt.float32r
F32 = mybir.dt.float32
AF = mybir.ActivationFunctionType

# Problem constants (hardcoded per spec nn_FFNwMoE_74380243632567)
B, S, D = 4, 2048, 2048
E, TOPK, H, SHARED = 8, 2, 1368, 1
AUX_COEF = 0.01
T = B * S                      # 8192 tokens
ND = D // 128                  # 16 d-tiles
NH = (H + 127) // 128          # 11 h-tiles (H padded 1368 -> 1408)
HPAD = NH * 128
NDC = D // 512                 # 4 output d-chunks
NCORES = 8
CB = T // NCORES               # shared-expert slice per core
TBMAX = 1280


def _block_list(CA, CB):
    blocks = []
    for ph, (start, size) in enumerate(((0, CA), (CA, CB))):
        off = start
        while off < start + size:
            rem = start + size - off
            TB = 1024 if rem > TBMAX else rem
            blocks.append((off, TB, ph))
            off += TB
    # Big blocks first within each phase: boundaries then always sit after a
    # long phase-3 window that hides the next block's x/weight prefetch.
    blocks.sort(key=lambda b: (b[2], -b[1]))
    out = []
    flat_off = 0
    for (t0, TB, ph) in blocks:
        out.append((t0, TB, ph, flat_off))
        flat_off += 128 * ND * TB
    return out


def _build_moe_kernel(CA, CB):
    """One SPMD Bass program; per-core data arrives via in_maps."""
    CTOT = CA + CB
    NTTG = CTOT // 128
    nc = bacc.Bacc("TRN2", target_bir_lowering=False, debug=False,
                   num_devices=NCORES)

    # xt: block-contiguous flat layout; per block [128, ND, TB] with
    # partition-major contiguity (per-partition run = ND*TB*4 bytes).
    xt = nc.dram_tensor("xt", [128 * ND * CTOT], F32R, kind="ExternalInput").ap()
    w13A = nc.dram_tensor("w13A", [NH, 128, ND, 256], F32R, kind="ExternalInput").ap()
    w2A = nc.dram_tensor("w2A", [NDC, 128, NH, 512], F32R, kind="ExternalInput").ap()
    w13B = nc.dram_tensor("w13B", [NH, 128, ND, 256], F32R, kind="ExternalInput").ap()
    w2B = nc.dram_tensor("w2B", [NDC, 128, NH, 512], F32R, kind="ExternalInput").ap()
    cw = nc.dram_tensor("cw", [128, NTTG], F32, kind="ExternalInput").ap()
    y = nc.dram_tensor("y", [CTOT, D], F32, kind="ExternalOutput").ap()

    blocks = _block_list(CA, CB)

    with tile.TileContext(nc) as tc, ExitStack() as ctx:
        xt_pool = ctx.enter_context(tc.tile_pool(name="xtp", bufs=1))
        h_pool = ctx.enter_context(tc.tile_pool(name="hp", bufs=1))
        w13_pool = ctx.enter_context(tc.tile_pool(name="w13p", bufs=3))
        w2_pool = ctx.enter_context(tc.tile_pool(name="w2p", bufs=2))
        y_pool = ctx.enter_context(tc.tile_pool(name="yp", bufs=2))
        cw_pool = ctx.enter_context(tc.tile_pool(name="cwp", bufs=1))
        psa_pool = ctx.enter_context(tc.tile_pool(name="psa", bufs=3, space="PSUM"))
        psb_pool = ctx.enter_context(tc.tile_pool(name="psb", bufs=3, space="PSUM"))
        psy_pool = ctx.enter_context(tc.tile_pool(name="psy", bufs=2, space="PSUM"))

        cw_all = cw_pool.tile([128, NTTG], F32, tag="cw")

        first_block = True
        for (t0, TB, ph, flat_off) in blocks:
            w13X = w13A if ph == 0 else w13B
            w2X = w2A if ph == 0 else w2B
            ntt = TB // 128
            subs = []
            off = 0
            while off < TB:
                subs.append((off, min(512, TB - off)))
                off += 512

            xt_sb = xt_pool.tile([128, ND, TBMAX], F32R, tag="xt")
            xt_blk = xt[flat_off:flat_off + 128 * ND * TB].rearrange(
                "(p kd t) -> p kd t", p=128, kd=ND)
            half = ND // 2
            q = ND // 4
            w13_first = (w13_pool.tile([128, ND // 2, 256], F32R, tag="w13",
                                       name="w13_f0"),
                         w13_pool.tile([128, ND // 2, 256], F32R, tag="w13",
                                       name="w13_f1"))
            if first_block:
                # startup critical path: first 512-token slice + h0 weights
                # land first, so PE starts after ~3.5MB instead of the whole
                # block; later blocks prefetch under the previous phase 3.
                s0 = min(512, TB)
                nc.sync.dma_start(xt_sb[:, :, :s0], xt_blk[:, :, :s0])
                nc.sync.dma_start(w13_first[0][:], w13X[0, :, :half, :])
                nc.sync.dma_start(w13_first[1][:], w13X[0, :, half:, :])
                off512 = s0
                while off512 < TB:
                    e512 = min(off512 + 512, TB)
                    nc.sync.dma_start(xt_sb[:, :, off512:e512],
                                      xt_blk[:, :, off512:e512])
                    off512 = e512
                nc.sync.dma_start(cw_all[:], cw)
                first_block = False
            else:
                nc.sync.dma_start(xt_sb[:, :q, :TB], xt_blk[:, :q, :])
                nc.sync.dma_start(w13_first[0][:], w13X[0, :, :half, :])
                for qi in range(1, 4):
                    nc.sync.dma_start(xt_sb[:, qi * q:(qi + 1) * q, :TB],
                                      xt_blk[:, qi * q:(qi + 1) * q, :])
                nc.sync.dma_start(w13_first[1][:], w13X[0, :, half:, :])

            h_sb = h_pool.tile([128, NH, TBMAX], F32R, tag="hsb")

            # phase 1: aT/bT[h, t] accumulation over d-tiles; h = silu(a)*b
            for h in range(NH):
                if h == 0:
                    w13_lo, w13_hi = w13_first
                else:
                    w13_lo = w13_pool.tile([128, ND // 2, 256], F32R, tag="w13")
                    w13_hi = w13_pool.tile([128, ND // 2, 256], F32R, tag="w13")
                    nc.sync.dma_start(w13_lo[:], w13X[h, :, :half, :])
                    nc.sync.dma_start(w13_hi[:], w13X[h, :, half:, :])

                def w13s(kd, j0, j1):
                    t = w13_lo if kd < half else w13_hi
                    return t[:, kd % half, j0:j1]

                for (so, sw) in subs:
                    psa = psa_pool.tile([128, 512], F32, tag="psa")
                    psb = psb_pool.tile([128, 512], F32, tag="psb")
                    for kd in range(ND):
                        nc.tensor.matmul(psa[:, :sw], w13s(kd, 0, 128),
                                         xt_sb[:, kd, so:so + sw],
                                         start=(kd == 0), stop=(kd == ND - 1))
                    for kd in range(ND):
                        nc.tensor.matmul(psb[:, :sw], w13s(kd, 128, 256),
                                         xt_sb[:, kd, so:so + sw],
                                         start=(kd == 0), stop=(kd == ND - 1))
                    hs = h_sb[:, h, so:so + sw]
                    nc.scalar.activation(hs, psa[:, :sw], AF.Sigmoid)
                    nc.vector.tensor_mul(hs, hs, psa[:, :sw])
                    nc.vector.tensor_mul(hs, hs, psb[:, :sw])

            # phase 3: y[t, d] = hT.T @ w2T over h-tiles, scaled by cw[t]
            for dc in range(NDC):
                w2_sb = w2_pool.tile([128, NH, 512], F32R, tag="w2")
                hh = NH // 2
                nc.sync.dma_start(w2_sb[:, :hh, :], w2X[dc, :, :hh, :])
                nc.sync.dma_start(w2_sb[:, hh:, :], w2X[dc, :, hh:, :])
                for tt in range(ntt):
                    psy = psy_pool.tile([128, 512], F32, tag="psy")
                    for h in range(NH):
                        nc.tensor.matmul(psy[:],
                                         h_sb[:, h, tt * 128:(tt + 1) * 128],
                                         w2_sb[:, h, :],
                                         start=(h == 0), stop=(h == NH - 1))
                    y_sb = y_pool.tile([128, 512], F32, tag="y")
                    ttg = t0 // 128 + tt
                    nc.vector.tensor_scalar_mul(y_sb[:], psy[:],
                                                cw_all[:, ttg:ttg + 1])
                    nc.sync.dma_start(
                        y[t0 + tt * 128:t0 + (tt + 1) * 128,
                          dc * 512:(dc + 1) * 512], y_sb[:])

    nc.compile()
    return nc


_KERNEL_CACHE = {}


def _get_kernel(CA):
    key = (CA, CB)
    if key not in _KERNEL_CACHE:
        _KERNEL_CACHE[key] = _build_moe_kernel(CA, CB)
    return _KERNEL_CACHE[key]


# ---------------- host-side packing ----------------

def _pack_w13(w1, w3):
    """w1, w3: [H, D] -> [NH, 128, ND, 256] with w13[h,p,kd,j]=w1p[h*128+j, kd*128+p]."""
    w1p = np.zeros((HPAD, D), np.float32)
    w1p[:H] = w1
    w3p = np.zeros((HPAD, D), np.float32)
    w3p[:H] = w3
    a = w1p.reshape(NH, 128, ND, 128).transpose(0, 3, 2, 1)
    b = w3p.reshape(NH, 128, ND, 128).transpose(0, 3, 2, 1)
    return np.ascontiguousarray(np.concatenate([a, b], axis=3))


def _pack_w2(w2):
    """w2: [D, H] -> [NDC, 128, NH, 512] with w2t[dc,p,h,j] = w2[dc*512+j, h*128+p]."""
    w2tp = np.zeros((HPAD, D), np.float32)
    w2tp[:H] = w2.T
    return np.ascontiguousarray(
        w2tp.reshape(NH, 128, NDC, 512).transpose(2, 1, 0, 3))


def _pack_xt(xcols, CA):
    """xcols: [CTOT, D] -> block-contiguous flat [128*ND*CTOT]:
    per block, layout [p, kd, t_local] with xt[p,kd,t]=xcols[t, kd*128+p]."""
    CTOT = xcols.shape[0]
    out = np.empty(128 * ND * CTOT, np.float32)
    for (t0, TB, _ph, flat_off) in _block_list(CA, CTOT - CA):
        blk = xcols[t0:t0 + TB].reshape(TB, ND, 128).transpose(2, 1, 0)
        out[flat_off:flat_off + 128 * ND * TB] = blk.reshape(-1)
    return out


def _routing(xf, router_w):
    """Replicate the reference's router math on CPU via jax (bit-matching ops).

    Returns (topk_idx [T, K] int, cw [T, E] f32, aux_loss f32 scalar).
    """
    import jax
    import jax.numpy as jnp
    cpu = jax.devices('cpu')[0]
    with jax.default_device(cpu):
        xj = jnp.asarray(xf)
        rwj = jnp.asarray(router_w)
        logits = xj @ rwj.T
        probs = jax.nn.softmax(logits, axis=-1)
        _, topk_idx = jax.lax.top_k(logits, TOPK)
        topk_p, _ = jax.lax.top_k(probs, TOPK)
        density = jax.nn.one_hot(topk_idx[:, 0], E, dtype=jnp.float32).mean(0)
        aux_loss = AUX_COEF * jnp.sum(density * probs.mean(0)) * E
        topk_idx = np.asarray(topk_idx)
        topk_p = np.asarray(topk_p)
        aux_loss = np.asarray(aux_loss)
    cwf = np.zeros((xf.shape[0], E), np.float32)
    np.add.at(cwf, (np.arange(xf.shape[0])[:, None], topk_idx), topk_p)
    return topk_idx, cwf, aux_loss


def kernel(x, router_w, w1, w2, w3, sw1, sw2, sw3, _run_opts=None):
    x = np.asarray(x, dtype=np.float32)
    router_w = np.asarray(router_w, dtype=np.float32)
    w1 = np.asarray(w1, dtype=np.float32)
    w2 = np.asarray(w2, dtype=np.float32)
    w3 = np.asarray(w3, dtype=np.float32)
    sw1 = np.asarray(sw1, dtype=np.float32)
    sw2 = np.asarray(sw2, dtype=np.float32)
    sw3 = np.asarray(sw3, dtype=np.float32)

    xf = x.reshape(T, D)
    topk_idx, cwf, aux_loss = _routing(xf, router_w)

    idx = [np.nonzero((topk_idx == e).any(axis=1))[0] for e in range(E)]
    counts = np.array([len(i) for i in idx])
    CA = max(256, int(-(-counts.max() // 256)) * 256)
    CTOT = CA + CB

    nc = _get_kernel(CA)

    w13B = _pack_w13(sw1[0], sw3[0])
    w2B = _pack_w2(sw2[0])

    in_maps = []
    for e in range(E):
        xcols = np.zeros((CTOT, D), np.float32)
        xcols[:counts[e]] = xf[idx[e]]
        xcols[CA:] = xf[e * CB:(e + 1) * CB]
        cw_col = np.zeros(CTOT, np.float32)
        cw_col[:counts[e]] = cwf[idx[e], e]
        cw_col[CA:] = 1.0
        in_maps.append({
            "xt": _pack_xt(xcols, CA),
            "w13A": _pack_w13(w1[e], w3[e]),
            "w2A": _pack_w2(w2[e]),
            "w13B": w13B,
            "w2B": w2B,
            "cw": np.ascontiguousarray(cw_col.reshape(CTOT // 128, 128).T),
        })

    run_opts = _run_opts or {}
    res = run_bass_kernel_spmd(nc, in_maps, core_ids=list(range(NCORES)),
                               **run_opts)

    out = np.zeros((T, D), np.float32)
    for e in range(E):
        ye = res.results[e]["y"]
        out[idx[e]] += ye[:counts[e]]
        out[e * CB:(e + 1) * CB] += ye[CA:]

    if run_opts:
        kernel._last_result = res
    return out.reshape(B, S, D), aux_loss
